# revision 1
# baseline (speedup 1.0000x reference)
"""DistGAT on 8 TRN2 NeuronCores (Bass/Tile).

Two-layer GAT (4 heads x 64 -> ELU -> 1 head x 40, head-mean) over a
50000-node / 800000-edge graph.

Strategy
--------
Graph partitioned by dst node: core c owns dst rows [c*6250, (c+1)*6250).
Edges are bucketed by (owner, dst-group of 128 rows, src-half) and sorted
by local dst on the host; segment softmax + segment sums become dense
128x128 indicator matmuls accumulated in PSUM.

The per-edge feature fetch is the memory bottleneck (random ~1KB rows;
the HW is row-access-bound at ~14ns/row on one SWDGE queue), so rows are
fetched with the CounterMachine-accelerated `dma_gather` round-robined
over 4 SWDGE queues (~3x). dma_gather indexes are int16, so each layer's
feature table is split into two overlapping halves (lo: rows [0,25088),
hi: rows [24960,end)) and each dst-group's edges are bucketed lo/hi —
all index arithmetic stays < 32768.

Feature tables are bf16 (row bytes are nearly free vs row count) with
the f32 attention logit `el` bit-packed into two bf16 slots per head.
Layer-0 table z0|el0 = x @ [W0 | W0@attn_l0] is computed replicated on
every core (bf16 matmul, f32 accumulate); er0 per dst row is recomputed
f32-exact from a per-core x slice. Layer-1 table is built distributed
and exchanged with one AllGather (~1.6 MB/core).

Host-side work is index bookkeeping only (bucket/sort/pad of edge ids);
all feature compute runs on the NeuronCores.
"""
import math
import sys

sys.path.insert(0, "/opt/trn_rl_repo")

import numpy as np
import ml_dtypes

import concourse.bass as bass
import concourse.bacc as bacc
import concourse.tile as tile
import concourse.mybir as mybir
from concourse import bass_utils
from concourse.masks import make_identity

# ---------------------------------------------------------------- constants
P = 128
N = 50000
E = 800000
F = 256
H0, D0 = 4, 64
C = 40
NEG = 0.2
M = 8
NC_ROWS = N // M            # 6250
G = (NC_ROWS + P - 1) // P  # 49 dst groups per core
NPAD = G * P                # 6272
NX = ((N + P - 1) // P) * P  # 50048 table rows (layer 0)
NXT = NX // P               # 391 tiles

# src-half split (int16 index range for dma_gather)
LO_ROWS = 25088             # zg_lo covers global rows [0, 25088)
HI_BASE = 24960             # zg_hi covers global rows [24960, 50048)
HI_ROWS = NX - HI_BASE      # 25088
L1_LO_ROWS = 25176          # z1 lo: gathered-row idx [0, 25176)
L1_HI_BASE = 25088          # z1 hi: idx - 25088, rows [25088, 50176)
L1_HI_ROWS = M * NPAD - L1_HI_BASE

W0T = 384                   # layer-0 table row (bf16): z 256 | el f32x4 | pad
W1T = 128                   # layer-1 table row (bf16): z1 40 | el1 f32x1 | pad
NS0 = F + H0                # 260: matmul rhs/psum width [a*z | a]
NS1 = C + 1                 # 41

F32 = mybir.dt.float32
BF16 = mybir.dt.bfloat16
I32 = mybir.dt.int32
I16 = mybir.dt.int16

_CACHE = {}
PHASEA_COPY = 'act'


# ---------------------------------------------------------------- host prep
def _edge_prep(src, dst):
    """Bucket edges by (dst owner, dst group, src half); sort; pad each
    bucket to T_H tiles of 128."""
    src = np.asarray(src).astype(np.int64)
    dst = np.asarray(dst).astype(np.int64)
    owner = dst // NC_ROWS
    dstloc = dst - owner * NC_ROWS
    half = (src >= LO_ROWS).astype(np.int64)   # 0 = lo, 1 = hi

    key = (owner * G + dstloc // P) * 2 + half
    counts = np.bincount(key, minlength=M * G * 2)
    T_H = max(1, int(math.ceil(counts.max() / P)))
    TPG = 2 * T_H               # tiles per group
    NT_ALL = G * TPG
    EP = NT_ALL * P

    src_owner = src // NC_ROWS
    g1 = src_owner * NPAD + (src - src_owner * NC_ROWS)  # layer-1 row id
    idx0 = np.where(half == 0, src, src - HI_BASE)
    idx1 = np.where(half == 0, g1, g1 - L1_HI_BASE)

    order = np.argsort(key, kind="stable")
    k_s = key[order]
    starts = np.zeros(M * G * 2 + 1, np.int64)
    np.cumsum(counts, out=starts[1:])
    slot = np.arange(E) - starts[k_s]
    core_i = k_s // (G * 2)
    g_i = (k_s // 2) % G
    h_i = k_s % 2
    pos = g_i * (TPG * P) + h_i * (T_H * P) + slot

    srcg0 = np.zeros((M, EP), np.int16)
    srcg1 = np.zeros((M, EP), np.int16)
    dstrel = np.full((M, EP), -1.0, np.float32)
    srcg0[core_i, pos] = idx0[order].astype(np.int16)
    srcg1[core_i, pos] = idx1[order].astype(np.int16)
    dstrel[core_i, pos] = (dstloc[order] % P).astype(np.float32)

    # int16 colmats with the 8x16-partition bank replication dma_gather wants:
    # bank k, partition 16k + i%16, col t*8 + i//16  <- idx of edge i of tile t
    def colmat16(a2):
        a3 = a2.reshape(M, NT_ALL, 8, 16)            # [c, t, i//16, i%16]
        out = np.zeros((M, 128, NT_ALL * 8), np.int16)
        for k in range(8):
            out[:, 16 * k:16 * (k + 1), :] = (
                a3.transpose(0, 3, 1, 2).reshape(M, 16, NT_ALL * 8))
        return np.ascontiguousarray(out)

    dstrel_cm = np.ascontiguousarray(
        dstrel.reshape(M, NT_ALL, P).transpose(0, 2, 1)
    ).astype(ml_dtypes.bfloat16)

    drel_rep = np.broadcast_to(
        dstrel.astype(ml_dtypes.bfloat16)[:, None, :], (M, P, NT_ALL * P))
    return T_H, NT_ALL, colmat16(srcg0), colmat16(srcg1), dstrel_cm, drel_rep


# ---------------------------------------------------------------- program
def _build(T_H, add_bias0, add_bias1, reps=1, mode='full', sim1=False):
    TPG = 2 * T_H
    NT_ALL = G * TPG
    nc = bacc.Bacc("TRN2", target_bir_lowering=False, debug=False,
                   enable_asserts=False, num_devices=(1 if sim1 else M),
                   num_swdge_queues=4)

    xin = nc.dram_tensor("xin", [NX, F], BF16, kind="ExternalInput").ap()
    xownT = nc.dram_tensor("xownT", [F, NPAD], BF16, kind="ExternalInput").ap()
    w0cat = nc.dram_tensor("w0cat", [F, F + H0], BF16, kind="ExternalInput").ap()
    wr0 = nc.dram_tensor("wr0", [F, H0], BF16, kind="ExternalInput").ap()
    w1cat = nc.dram_tensor("w1cat", [F, C + 2], F32, kind="ExternalInput").ap()
    iotarep = nc.dram_tensor("iotarep", [P, P], BF16, kind="ExternalInput").ap()
    srcg0 = nc.dram_tensor("srcg0", [P, NT_ALL * 8], I16, kind="ExternalInput").ap()
    srcg1 = nc.dram_tensor("srcg1", [P, NT_ALL * 8], I16, kind="ExternalInput").ap()
    dstrel = nc.dram_tensor("dstrel", [P, NT_ALL], BF16, kind="ExternalInput").ap()
    drel_rep = nc.dram_tensor("drel_rep", [P, NT_ALL * P], BF16,
                              kind="ExternalInput").ap()
    bias0 = nc.dram_tensor("bias0", [P, F], F32, kind="ExternalInput").ap()
    bias1 = nc.dram_tensor("bias1", [P, C], F32, kind="ExternalInput").ap()
    out = nc.dram_tensor("out", [NC_ROWS, C], F32, kind="ExternalOutput").ap()

    with tile.TileContext(nc) as tc:
        cpool = tc.alloc_tile_pool(name="const", bufs=1)
        dpool = tc.alloc_tile_pool(name="dram", bufs=1, space="DRAM")
        ppool = tc.alloc_tile_pool(name="psum", bufs=2, space="PSUM")
        wpool = tc.alloc_tile_pool(name="work", bufs=3)

        for _rep in range(reps):
            # ---- constants
            w0a = cpool.tile([P, F + H0], BF16)
            w0b = cpool.tile([P, F + H0], BF16)
            nc.sync.dma_start(w0a[:], w0cat[0:P, :])
            nc.sync.dma_start(w0b[:], w0cat[P:F, :])
            wr0a = cpool.tile([P, H0], BF16)
            wr0b = cpool.tile([P, H0], BF16)
            nc.sync.dma_start(wr0a[:], wr0[0:P, :])
            nc.sync.dma_start(wr0b[:], wr0[P:F, :])
            w1a = cpool.tile([P, C + 2], F32)
            w1b = cpool.tile([P, C + 2], F32)
            nc.sync.dma_start(w1a[:], w1cat[0:P, :])
            nc.sync.dma_start(w1b[:], w1cat[P:F, :])
            iot = cpool.tile([P, P], BF16)
            nc.sync.dma_start(iot[:], iotarep[:])
            iotacol = cpool.tile([P, 1], F32)
            nc.gpsimd.iota(iotacol[:], pattern=[[0, 1]], base=0,
                           channel_multiplier=1,
                           allow_small_or_imprecise_dtypes=True)
            srcg0_sb = cpool.tile([P, NT_ALL * 8], I16)
            nc.sync.dma_start(srcg0_sb[:], srcg0[:])
            srcg1_sb = cpool.tile([P, NT_ALL * 8], I16)
            nc.sync.dma_start(srcg1_sb[:], srcg1[:])
            dstrel_sb = cpool.tile([P, NT_ALL], BF16)
            nc.sync.dma_start(dstrel_sb[:], dstrel[:])
            ident_b = cpool.tile([P, P], BF16)
            make_identity(nc, ident_b)
            ident_f = cpool.tile([P, P], F32)
            make_identity(nc, ident_f)
            xoT_a = cpool.tile([P, NPAD], BF16)
            xoT_b = cpool.tile([P, NPAD], BF16)
            nc.sync.dma_start(xoT_a[:], xownT[0:P, :])
            nc.sync.dma_start(xoT_b[:], xownT[P:F, :])
            er0_sb = cpool.tile([P, G, 2 * H0], BF16)
            er1_sb = cpool.tile([P, G, 2], BF16)
            if mode == 'nogather':
                zel_dummy = cpool.tile([P, 1, W0T], BF16)
                nc.gpsimd.memset(zel_dummy[:], 0.25)
            if add_bias0:
                b0_sb = cpool.tile([P, F], F32)
                nc.sync.dma_start(b0_sb[:], bias0[:])
            if add_bias1:
                b1_sb = cpool.tile([P, C], F32)
                nc.sync.dma_start(b1_sb[:], bias1[:])

            # ---- DRAM tables
            zg_lo = dpool.tile([LO_ROWS, W0T], BF16)
            zg_hi = dpool.tile([HI_ROWS, W0T], BF16)
            z1loc = dpool.tile([NPAD, W1T], BF16)
            z1gat = dpool.tile([M * NPAD, W1T], BF16, addr_space="Shared")
            z1g_hi = dpool.tile([L1_HI_ROWS, W1T], BF16)

            # ============== phase A: z0ext = x @ [W0|wl0] replicated, bf16
            for ci in range(NX // 512 + 1 if mode != 'gatheronly' else 0):
                r0 = ci * 512
                nt = min(512, NX - r0)
                if nt <= 0:
                    break
                xt0 = wpool.tile([P, nt], BF16, tag="xt0", bufs=3, padded_shape=[P, 512])
                xt1 = wpool.tile([P, nt], BF16, tag="xt1", bufs=3, padded_shape=[P, 512])
                nc.sync.dma_start(xt0[:], xin[r0:r0 + nt, 0:P], transpose=True)
                nc.sync.dma_start(xt1[:], xin[r0:r0 + nt, P:F], transpose=True)
                nsub = nt // P
                z_sb = wpool.tile([P, 4, W0T], BF16, tag="z_sb", bufs=3,
                                  padded_shape=[P, 4, W0T])
                for s in range(nsub):
                    zps = ppool.tile([P, F + H0], F32, tag="outp")
                    nc.tensor.matmul(zps[:], lhsT=xt0[:, s * P:(s + 1) * P],
                                     rhs=w0a[:], start=True, stop=False)
                    nc.tensor.matmul(zps[:], lhsT=xt1[:, s * P:(s + 1) * P],
                                     rhs=w0b[:], start=False, stop=True)
                    nc.scalar.copy(z_sb[:, s, 0:F], zps[:, 0:F])
                    nc.vector.tensor_copy(
                        out=z_sb[:, s, F:F + 2 * H0].bitcast(F32),
                        in_=zps[:, F:F + H0])
                rr = r0
                lo_hi = min(rr + nt, LO_ROWS)
                if lo_hi > rr:
                    s0, s1 = 0, (lo_hi - rr) // P
                    nc.sync.dma_start(
                        zg_lo[rr:lo_hi, :].rearrange("(k p) w -> p k w", p=P),
                        z_sb[:, s0:s1, :])
                hi_lo = max(rr, HI_BASE)
                if rr + nt > hi_lo:
                    s0 = (hi_lo - rr) // P
                    nc.sync.dma_start(
                        zg_hi[hi_lo - HI_BASE:rr + nt - HI_BASE, :]
                        .rearrange("(k p) w -> p k w", p=P),
                        z_sb[:, s0:nsub, :])

            # ============== phase A2: er0 per own dst row (bf16 matmul)
            for g in range(G if mode != 'gatheronly' else 0):
                erp0 = ppool.tile([P, H0], F32, tag="erp")
                nc.tensor.matmul(erp0[:], lhsT=xoT_a[:, g * P:(g + 1) * P],
                                 rhs=wr0a[:], start=True, stop=False)
                nc.tensor.matmul(erp0[:], lhsT=xoT_b[:, g * P:(g + 1) * P],
                                 rhs=wr0b[:], start=False, stop=True)
                nc.vector.tensor_copy(out=er0_sb[:, g, 0:H0], in_=erp0[:])
                nc.vector.tensor_tensor(out=er0_sb[:, g, H0:2 * H0],
                                        in0=erp0[:], in1=er0_sb[:, g, 0:H0],
                                        op=mybir.AluOpType.subtract)

            # ============== per-layer edge aggregation (batched tiles)
            def edge_layer(layer):
                if layer == 0:
                    tabs, wrow, nfeat, nh = (zg_lo, zg_hi), W0T, F, H0
                    offs = srcg0_sb
                else:
                    tabs, wrow, nfeat, nh = (z1gat[0:L1_LO_ROWS, :], z1g_hi[:]), W1T, C, 1
                    offs = srcg1_sb
                nsum = nfeat + nh
                # batches per half: e.g. T_H=9 -> [4, 4, 1]
                bat = []
                r = T_H
                while r > 0:
                    bat.append(min(4, r))
                    r -= bat[-1]
                qctr = [0]

                for g in range(G):
                    outp = ppool.tile([P, nsum], F32, tag="outp")
                    ergrp = (er0_sb[:, g, :] if layer == 0
                             else er1_sb[:, g, :])
                    rr_grp = wpool.tile([P, TPG, P], BF16, tag=f"rr{layer}", bufs=2)
                    if mode != 'gatheronly':
                        nc.sync.dma_start(
                            rr_grp[:],
                            drel_rep[:, g * TPG * P:(g + 1) * TPG * P]
                            .rearrange("p (b e) -> p b e", e=P))
                    n_mm = 0
                    for half in range(2):
                        j0 = half * T_H
                        boff = 0
                        for B in bat:
                            t = g * TPG + j0 + boff
                            BP = B * P
                            zel = wpool.tile([P, B, wrow], BF16,
                                             tag=f"zel{layer}", bufs=6,
                                             padded_shape=[P, 4, wrow])
                            if mode != 'nogather':
                                nc.gpsimd.dma_gather(
                                    out_ap=zel[:],
                                    in_ap=tabs[half],
                                    idxs_ap=offs[:, t * 8:(t + B) * 8],
                                    num_idxs=BP, num_idxs_reg=BP,
                                    elem_size=wrow,
                                    queue_num=qctr[0] % 4)
                            qctr[0] += 1
                            if mode == 'gatheronly':
                                boff += B
                                n_mm += B
                                continue
                            # S_all [P, B*128] bf16 and S_T_all [P, B*128] f32
                            smat = wpool.tile([P, B, P], BF16,
                                              tag=f"smat{layer}", bufs=4,
                                              padded_shape=[P, 4, P])
                            nc.vector.tensor_tensor(
                                out=smat[:],
                                in0=dstrel_sb[:, t:t + B].unsqueeze(-1)
                                    .to_broadcast([P, B, P]),
                                in1=iot[:].unsqueeze(1).to_broadcast([P, B, P]),
                                op=mybir.AluOpType.is_equal)
                            jb = j0 + boff
                            st_all = wpool.tile([P, B, P], BF16,
                                                tag=f"st{layer}", bufs=4,
                                                padded_shape=[P, 4, P])
                            nc.vector.tensor_tensor(
                                out=st_all[:],
                                in0=iotacol[:].unsqueeze(-1).to_broadcast([P, B, P]),
                                in1=rr_grp[:, jb:jb + B, :],
                                op=mybir.AluOpType.is_equal)
                            # er broadcast to edges: per tile matmul into one psum
                            erp = ppool.tile([P, B, 2 * nh], F32, tag="erp",
                                             padded_shape=[P, 4, 2 * nh])
                            for j in range(B):
                                nc.tensor.matmul(erp[:, j, :], lhsT=st_all[:, j, :],
                                                 rhs=ergrp, start=True, stop=True)
                            # logits -> leaky -> exp -> a
                            el = zel[:, :, nfeat:nfeat + 2 * nh].bitcast(F32)
                            lg = wpool.tile([P, B, nh], F32, tag=f"lg{layer}",
                                            bufs=4, padded_shape=[P, 4, nh])
                            nc.vector.tensor_tensor(out=lg[:], in0=el,
                                                    in1=erp[:, :, 0:nh],
                                                    op=mybir.AluOpType.add)
                            nc.vector.tensor_tensor(out=lg[:], in0=lg[:],
                                                    in1=erp[:, :, nh:2 * nh],
                                                    op=mybir.AluOpType.add)
                            lk = wpool.tile([P, B, nh], F32, tag=f"lk{layer}",
                                            bufs=4, padded_shape=[P, 4, nh])
                            nc.vector.tensor_scalar(out=lk[:], in0=lg[:],
                                                    scalar1=NEG, scalar2=None,
                                                    op0=mybir.AluOpType.mult)
                            nc.vector.tensor_tensor(out=lk[:], in0=lg[:], in1=lk[:],
                                                    op=mybir.AluOpType.max)
                            rhs_t = wpool.tile([P, B, nsum], BF16,
                                               tag=f"rhs{layer}", bufs=4,
                                               padded_shape=[P, 4, nsum])
                            nc.scalar.activation(rhs_t[:, :, nfeat:nsum], lk[:],
                                                 mybir.ActivationFunctionType.Exp)
                            nc.vector.tensor_tensor(
                                out=rhs_t[:, :, 0:nfeat]
                                    .rearrange("p b (h d) -> p b h d", h=nh),
                                in0=zel[:, :, 0:nfeat]
                                    .rearrange("p b (h d) -> p b h d", h=nh),
                                in1=rhs_t[:, :, nfeat:nsum].unsqueeze(-1)
                                    .to_broadcast([P, B, nh, nfeat // nh]),
                                op=mybir.AluOpType.mult)
                            for j in range(B):
                                nc.tensor.matmul(outp[:], lhsT=smat[:, j, :],
                                                 rhs=rhs_t[:, j, :],
                                                 start=(n_mm == 0),
                                                 stop=(n_mm == TPG - 1))
                                n_mm += 1
                            boff += B
                    if mode == 'gatheronly':
                        continue

                    # ---- group finalize
                    if mode == 'gatheronly':
                        continue
                    s4 = wpool.tile([P, nh], F32, tag=f"s4{layer}", bufs=2)
                    nc.vector.tensor_scalar(out=s4[:], in0=outp[:, nfeat:nsum],
                                            scalar1=1e-30, scalar2=None,
                                            op0=mybir.AluOpType.max)
                    rec = wpool.tile([P, nh], F32, tag=f"rec{layer}", bufs=2)
                    nc.vector.reciprocal(rec[:], s4[:])
                    if layer == 0:
                        y = wpool.tile([P, F], F32, tag="y", bufs=2)
                        nc.vector.tensor_tensor(
                            out=y[:].rearrange("p (h d) -> p h d", h=nh),
                            in0=outp[:, 0:F].rearrange("p (h d) -> p h d", h=nh),
                            in1=rec[:].unsqueeze(-1).to_broadcast([P, nh, D0]),
                            op=mybir.AluOpType.mult)
                        if add_bias0:
                            nc.vector.tensor_tensor(out=y[:], in0=y[:], in1=b0_sb[:],
                                                    op=mybir.AluOpType.add)
                        # ELU: max(y,0) + min(exp(y),1) - 1, in bf16 h
                        ey = wpool.tile([P, F], F32, tag="ey", bufs=2)
                        nc.scalar.activation(ey[:], y[:],
                                             mybir.ActivationFunctionType.Exp)
                        t1 = wpool.tile([P, F], F32, tag="t1", bufs=2)
                        nc.vector.tensor_scalar(out=t1[:], in0=ey[:],
                                                scalar1=1.0, scalar2=-1.0,
                                                op0=mybir.AluOpType.min,
                                                op1=mybir.AluOpType.add)
                        h = wpool.tile([P, F], BF16, tag="h", bufs=2)
                        nc.vector.tensor_scalar(out=h[:], in0=y[:], scalar1=0.0,
                                                scalar2=None, op0=mybir.AluOpType.max)
                        nc.vector.tensor_tensor(out=h[:], in0=h[:], in1=t1[:],
                                                op=mybir.AluOpType.add)
                        # z1ext = h @ [W1|wl1|wr1]  (f32 accumulate via f32 hT)
                        z1p = ppool.tile([P, C + 2], F32, tag="erp")
                        for half in range(2):
                            tp = ppool.tile([P, P], BF16, tag="stp")
                            nc.tensor.transpose(tp[:], h[:, half * P:(half + 1) * P],
                                                ident_b[:])
                            ht = wpool.tile([P, P], F32, tag="ht", bufs=2)
                            nc.vector.tensor_copy(out=ht[:], in_=tp[:])
                            nc.tensor.matmul(z1p[:], lhsT=ht[:],
                                             rhs=(w1a[:] if half == 0 else w1b[:]),
                                             start=(half == 0), stop=(half == 1))
                        z1_sb = wpool.tile([P, W1T], BF16, tag="z1sb", bufs=2)
                        nc.scalar.copy(z1_sb[:, 0:C], z1p[:, 0:C])
                        nc.scalar.copy(z1_sb[:, C:C + 2].bitcast(F32),
                                       z1p[:, C:C + 1])
                        nc.vector.tensor_copy(out=er1_sb[:, g, 0:1],
                                              in_=z1p[:, C + 1:C + 2])
                        nc.vector.tensor_tensor(out=er1_sb[:, g, 1:2],
                                                in0=z1p[:, C + 1:C + 2],
                                                in1=er1_sb[:, g, 0:1],
                                                op=mybir.AluOpType.subtract)
                        nc.sync.dma_start(z1loc[g * P:(g + 1) * P, :], z1_sb[:])
                    else:
                        o = wpool.tile([P, C], F32, tag="o", bufs=2)
                        nc.vector.tensor_scalar(out=o[:], in0=outp[:, 0:C],
                                                scalar1=rec[:], scalar2=None,
                                                op0=mybir.AluOpType.mult)
                        if add_bias1:
                            nc.vector.tensor_tensor(out=o[:], in0=o[:], in1=b1_sb[:],
                                                    op=mybir.AluOpType.add)
                        rows = min(P, NC_ROWS - g * P)
                        nc.sync.dma_start(out[g * P:g * P + rows, :], o[0:rows, :])

            if mode != 'phaseA':
                edge_layer(0)

            # ============== exchange z1ext; build hi half-table
            if mode in ('phaseA', 'noL1'):
                continue
            if sim1:
                nc.sync.dma_start(z1gat[0:NPAD, :], z1loc[:])
            else:
                nc.gpsimd.collective_compute(
                    "AllGather", mybir.AluOpType.bypass,
                    ins=[z1loc.opt()], outs=[z1gat.opt()],
                    replica_groups=[list(range(M))])
            nc.sync.dma_start(z1g_hi[:], z1gat[L1_HI_BASE:M * NPAD, :])

            edge_layer(1)


        for pool in (wpool, ppool, dpool, cpool):
            pool.release()

    nc.compile()
    return nc


# ---------------------------------------------------------------- entry
def _prepare(x, src, dst, W0, attn_l0, attn_r0, bias0, W1, attn_l1, attn_r1,
             bias1):
    x = np.asarray(x, np.float32)
    W0 = np.asarray(W0, np.float32)
    W1 = np.asarray(W1, np.float32)
    attn_l0 = np.asarray(attn_l0, np.float32)
    attn_r0 = np.asarray(attn_r0, np.float32)
    attn_l1 = np.asarray(attn_l1, np.float32)
    attn_r1 = np.asarray(attn_r1, np.float32)
    b0 = np.asarray(bias0, np.float32)
    b1 = np.asarray(bias1, np.float32)

    T_H, NT_ALL, srcg0_cm, srcg1_cm, dstrel_cm, drel_rep_cm = _edge_prep(src, dst)

    W0h = W0.reshape(F, H0, D0)
    wl0 = np.einsum("fhd,hd->fh", W0h, attn_l0)
    wr0 = np.einsum("fhd,hd->fh", W0h, attn_r0)
    w0cat = np.concatenate([W0, wl0], axis=1)            # [F, 260]
    wl1 = W1 @ attn_l1.reshape(C, 1)
    wr1 = W1 @ attn_r1.reshape(C, 1)
    w1cat = np.concatenate([W1, wl1, wr1], axis=1)       # [F, 42]

    xpad = np.zeros((NX, F), np.float32)
    xpad[:N] = x
    iotarep = np.broadcast_to(np.arange(P, dtype=np.float32),
                              (P, P)).astype(ml_dtypes.bfloat16)

    add_bias0 = bool(np.any(b0 != 0))
    add_bias1 = bool(np.any(b1 != 0))

    common = {
        "xin": xpad.astype(ml_dtypes.bfloat16),
        "w0cat": w0cat.astype(ml_dtypes.bfloat16),
        "wr0": wr0.astype(ml_dtypes.bfloat16),
        "w1cat": w1cat.astype(np.float32),
        "iotarep": np.ascontiguousarray(iotarep),
        "bias0": np.broadcast_to(b0, (P, F)).copy(),
        "bias1": np.broadcast_to(b1, (P, C)).copy(),
    }
    in_maps = []
    for c in range(M):
        m = dict(common)
        xo = np.zeros((NPAD, F), np.float32)
        xo[:NC_ROWS] = x[c * NC_ROWS:(c + 1) * NC_ROWS]
        m["xownT"] = np.ascontiguousarray(xo.T).astype(ml_dtypes.bfloat16)
        m["srcg0"] = srcg0_cm[c]
        m["srcg1"] = srcg1_cm[c]
        m["dstrel"] = dstrel_cm[c]
        m["drel_rep"] = np.ascontiguousarray(drel_rep_cm[c])
        in_maps.append(m)
    return T_H, add_bias0, add_bias1, in_maps


def kernel(**inputs):
    T_H, ab0, ab1, in_maps = _prepare(**inputs)
    key = (T_H, ab0, ab1)
    if key not in _CACHE:
        _CACHE[key] = _build(T_H, ab0, ab1)
    nc = _CACHE[key]
    res = bass_utils.run_bass_kernel_spmd(nc, in_maps, core_ids=list(range(M)))
    out = np.concatenate([r["out"] for r in res.results], axis=0)
    return out.astype(np.float32)

